# revision 1
# baseline (speedup 1.0000x reference)
"""Masked-softmax cross-entropy loss on 8 Trainium2 cores.

Math: for each target row t (16384 rows of length 4096):
  numer[t] = sum_j exp(x[t,j]/tau) over valid src cols j whose color == tgt color t
  denom[t] = sum_j exp(x[t,j]/tau) over valid src cols j
  p_gt = numer/denom, nll = -log(p_gt + eps), rows with numer==0 are masked out.
Segment/count aggregation (32 segments) happens on host - it touches 16K scalars.

Sharding: core c takes half a batch: batch c//2, row-half c%2 (2048 rows).
All rows on a core share one batch => one src color-id row.

Device pipeline per 256-row chunk (two 128-row tiles side by side):
  DMA (swdge):  load x chunk [128, 8192] f32 (contiguous 4MB)
  ScalarE:      et = exp(10*x) -> bf16, accum_out -> denom_all  (per tile)
  DVE (bf16 2x mode) per tile, fused compare-multiply-accumulate STTs:
      numer   = sum((src_id == tgt_id[t]) * et)
      invsum  = sum((src_id == -1)        * et)   (invalid-column mass)
Host: denom = denom_all - invsum.
Colors are mapped to small integer ids on host (exact byte equality), so a
bf16 equality compare on device reproduces the reference's exact color match.
src pad -> id -1, tgt pad -> id -2 (never matches anything valid).

Sync-wait budget: this walrus allows very few sem waits per instruction
(1 for STT/DMA/CTRL). Tiny same-engine "interposer" copies absorb
cross-engine waits, and the kernel-tail drain is split into one drain per
proc. Absorbers sit on cheap queues (scalar/vector copies ~80-300ns; pool
only absorbs for the loads it issues).
"""

import os
import numpy as np

B = 4
S_TGT = 8
L_TGT = 512
C = 4
N = 4096          # src columns (= 8*512), also total tgt rows per batch
P = 128
ROWS = 2048       # tgt rows per core (half a batch)
NTILES = ROWS // P    # 16 result tiles
TPC = 2               # tiles per DMA chunk
NCHUNK = NTILES // TPC
NBUF = 3              # chunk buffer depth (slot reuse distance)
NCORES = 8
PAD = -1.0
EPS = 1e-15

_NC_CACHE = {}


def _patch_split_drain():
    """Split the kernel-tail drain's sem waits across several drain
    instructions (walrus rejects >1 sync wait on one CTRL instruction)."""
    import concourse.tile as tile
    from concourse.vector_clock import ScopedClock, VectorClock

    if getattr(tile.TileContext, "_split_drain_patched", False):
        return

    def _drain_and_barrier(self, tick_clock, wait_clock):
        g = tick_clock.global_clock
        n = len(g)
        for base in range(n):
            vec = [g[i] if i == base else 0 for i in range(n)]
            if not any(vec):
                continue
            d = self.nc.sync.drain()
            wait_clock.add_sem_waits(d.ins, ScopedClock({None: VectorClock(vec)}))
        self.nc.all_engine_barrier()
        popped = self.nc._tile_sem_poison_stack.pop()
        assert popped is self._sem_poison
        self.nc.clear_and_free_semaphores(list(self.sems.allocated().values()))
        self.nc.all_engine_barrier()

    tile.TileContext._drain_and_barrier = _drain_and_barrier
    tile.TileContext._split_drain_patched = True


def _build_nc():
    import concourse.bass as bass
    import concourse.mybir as mybir
    import concourse.tile as tile
    from concourse.tile_rust import add_dep_helper
    from contextlib import ExitStack

    _patch_split_drain()
    nc = bass.Bass()
    f32 = mybir.dt.float32
    bf16 = mybir.dt.bfloat16
    NW = N * TPC  # chunk width in f32 elements
    x = nc.declare_dram_parameter("x", [ROWS, N], f32, isOutput=False)
    src_ids = nc.declare_dram_parameter("src_ids", [P, N], bf16, isOutput=False)
    tgt_ids = nc.declare_dram_parameter("tgt_ids", [P, NTILES], bf16,
                                        isOutput=False)
    numer = nc.declare_dram_parameter("numer", [P, NTILES], f32, isOutput=True)
    denall = nc.declare_dram_parameter("denall", [P, NTILES], f32, isOutput=True)
    invsum = nc.declare_dram_parameter("invsum", [P, NTILES], f32, isOutput=True)

    with tile.TileContext(nc) as tc:
        with ExitStack() as ctx:
            const_pool = ctx.enter_context(tc.tile_pool(name="const", bufs=1))
            x_pool = ctx.enter_context(tc.tile_pool(name="x", bufs=NBUF))
            e_pool = ctx.enter_context(tc.tile_pool(name="exps", bufs=NBUF))
            res_pool = ctx.enter_context(tc.tile_pool(name="res", bufs=1))

            sid = const_pool.tile([P, N], bf16)
            nc.sync.dma_start(sid[:], src_ids[:])
            tid = const_pool.tile([P, NTILES], bf16)
            nc.sync.dma_start(tid[:], tgt_ids[:])
            jpool = ctx.enter_context(tc.tile_pool(name="junk", bufs=1))
            junk = jpool.tile([P, N], bf16)
            res_n = res_pool.tile([P, NTILES], f32)
            res_d = res_pool.tile([P, NTILES], f32)
            res_i = res_pool.tile([P, NTILES], f32)

            # warm-up copies absorb the const-DMA waits per engine
            warm = res_pool.tile([P, 4], bf16)
            nc.vector.tensor_copy(warm[:, 0:1], sid[:, 0:1])
            nc.vector.tensor_copy(warm[:, 1:2], tid[:, 0:1])
            nc.scalar.copy(warm[:, 2:3], sid[:, 0:1])
            nc.gpsimd.tensor_copy(warm[:, 3:4], tid[:, 0:1])

            def scratch(prefix, dt_=f32):
                return [
                    res_pool.tile([P, 1], dt_, name=f"{prefix}{i}",
                                  tag=f"{prefix}{i}")
                    for i in range(NTILES)
                ]

            accn = scratch("an")
            accd = scratch("ad")
            acci = scratch("ai")
            sca, scc, scd, sce, scf, sch, sci = (
                scratch("sa"), scratch("scc"), scratch("sd"), scratch("se"),
                scratch("sf"), scratch("sh"), scratch("si"),
            )

            load_insts = []
            for ci in range(NCHUNK):
                xt = x_pool.tile([P, NW], f32)
                # pool-queue interposers: absorb the load's cross-engine
                # waits (scalar's reads of the recycled slot / the DMA lane
                # WAW) so the SWDGE DMACopy keeps a single sync wait
                pre = []
                if ci >= NBUF:
                    gA = nc.gpsimd.tensor_copy(
                        scd[ci][:], accd[(ci - NBUF) * TPC + TPC - 1][:]
                    )
                    pre.append(gA)
                    for k, old in enumerate(load_insts[ci - NBUF]):
                        gB = nc.gpsimd.tensor_copy(
                            (sce[ci] if k == 0 else scf[ci])[:], tid[:, 0:1]
                        )
                        add_dep_helper(
                            gB.ins, old.ins, sync=True,
                            reason="absorb DMA lane WAW",
                        )
                        pre.append(gB)
                lds = []
                base = ci * P * TPC
                for k in range(TPC):
                    ld = nc.gpsimd.dma_start(
                        xt[:, k * N:(k + 1) * N],
                        x[base + k * P:base + (k + 1) * P, :],
                    )
                    for g in pre:
                        add_dep_helper(
                            ld.ins, g.ins, sync=False,
                            reason="load ordered after wait absorber",
                        )
                    lds.append(ld)
                load_insts.append(lds)

                et = e_pool.tile([P, NW], bf16)
                for h in range(TPC):
                    i = ci * TPC + h
                    xs = xt[:, h * N:(h + 1) * N]
                    es = et[:, h * N:(h + 1) * N]

                    # scalar-side absorbers: DMA-lane wait + et-slot WAW
                    exp_deps = []
                    if h == 0:
                        exp_deps.append(nc.scalar.copy(scc[i][:], xt[:, 0:1]))
                    if ci >= NBUF:
                        exp_deps.append(
                            nc.scalar.copy(sca[i][:], accn[i - NBUF * TPC][:])
                        )
                    exp = nc.scalar.activation(
                        es, xs, mybir.ActivationFunctionType.Exp,
                        scale=10.0, accum_out=accd[i][:],
                    )
                    for d in exp_deps:
                        add_dep_helper(
                            exp.ins, d.ins, sync=False,
                            reason="exp ordered after wait absorber",
                        )

                    # DVE absorber for the et-slot WAW, then the two fused
                    # compare-multiply-accumulate STTs (junk out in-place)
                    spre = []
                    if i >= 1:
                        vC = nc.vector.tensor_copy(sch[i][:], accn[i - 1][:])
                        spre.append(vC)
                    # STT1 writes its junk to a shared scratch tile so STT2
                    # still sees the clean exp values; STT2 (last reader)
                    # junks in place over et
                    stt1 = nc.vector.scalar_tensor_tensor(
                        out=junk[:], in0=sid[:], scalar=tid[:, i:i + 1], in1=es,
                        op0=mybir.AluOpType.is_equal,
                        op1=mybir.AluOpType.mult,
                        accum_out=accn[i][:],
                    )
                    # direct masked denominator: no cancellation against the
                    # (free) exp-accumulated total, which breaks down for rows
                    # dominated by invalid-column mass
                    stt2 = nc.vector.scalar_tensor_tensor(
                        out=es, in0=sid[:], scalar=-1.0, in1=es,
                        op0=mybir.AluOpType.not_equal,
                        op1=mybir.AluOpType.mult,
                        accum_out=acci[i][:],
                    )
                    for g in spre:
                        add_dep_helper(
                            stt1.ins, g.ins, sync=False,
                            reason="STT1 ordered after WAW absorber",
                        )

            for i in range(NTILES):
                nc.vector.tensor_copy(res_n[:, i:i + 1], accn[i][:])
                nc.vector.tensor_copy(res_d[:, i:i + 1], accd[i][:])
                nc.vector.tensor_copy(res_i[:, i:i + 1], acci[i][:])
            nc.sync.dma_start(numer[:], res_n[:])
            nc.sync.dma_start(denall[:], res_d[:])
            nc.sync.dma_start(invsum[:], res_i[:])
    return nc


def _get_nc():
    key = (NBUF, TPC)
    if key not in _NC_CACHE:
        _NC_CACHE[key] = _build_nc()
    return _NC_CACHE[key]


def _color_ids(src, tgt):
    """Map each color row to a per-batch integer id via exact byte equality."""
    src_f = np.ascontiguousarray(src.reshape(B, -1, C))
    tgt_f = np.ascontiguousarray(tgt.reshape(B, -1, C))
    n_s = src_f.shape[1]
    src_ids = np.empty((B, n_s), np.float32)
    tgt_ids = np.empty((B, tgt_f.shape[1]), np.float32)
    for b in range(B):
        allc = np.ascontiguousarray(np.concatenate([src_f[b], tgt_f[b]], axis=0))
        view = allc.view([("", allc.dtype)] * C).reshape(-1)
        _, inv = np.unique(view, return_inverse=True)
        ids = inv.astype(np.float32)
        s_ids, t_ids = ids[:n_s].copy(), ids[n_s:].copy()
        s_ids[np.all(src_f[b] == PAD, axis=-1)] = -1.0
        t_ids[np.all(tgt_f[b] == PAD, axis=-1)] = -2.0
        src_ids[b], tgt_ids[b] = s_ids, t_ids
    return src_ids, tgt_ids


def kernel(seg_sim_map, seg_colors_src, seg_colors_tgt):
    import ml_dtypes
    from concourse.bass_utils import run_bass_kernel_spmd

    bf16 = ml_dtypes.bfloat16
    seg_sim_map = np.asarray(seg_sim_map, dtype=np.float32)
    src_ids, tgt_ids = _color_ids(
        np.asarray(seg_colors_src, np.float32), np.asarray(seg_colors_tgt, np.float32)
    )

    in_maps = []
    for c in range(NCORES):
        b, h = c // 2, c % 2
        rows = slice(h * ROWS, (h + 1) * ROWS)
        in_maps.append({
            "x": np.ascontiguousarray(seg_sim_map[b, rows, :]),
            "src_ids": np.ascontiguousarray(
                np.broadcast_to(src_ids[b].astype(bf16), (P, N))
            ),
            # [p, i] = id of row i*P + p
            "tgt_ids": np.ascontiguousarray(
                tgt_ids[b, rows].reshape(NTILES, P).T.astype(bf16)
            ),
        })

    trace = os.environ.get("KERNEL_PROFILE", "") == "1"
    nc = _get_nc()
    out = run_bass_kernel_spmd(nc, in_maps, list(range(NCORES)), trace=trace)
    if trace and out.exec_time_ns is not None:
        print(f"HW exec time: {out.exec_time_ns} ns")
        print(f"HW exec mean: {out.mean_exec_time_ns} ns")

    numer = np.empty((B, N), np.float32)
    denom = np.empty((B, N), np.float32)
    for c in range(NCORES):
        b, h = c // 2, c % 2
        rows = slice(h * ROWS, (h + 1) * ROWS)
        r = out.results[c]
        numer[b, rows] = r["numer"].T.reshape(ROWS)
        denom[b, rows] = r["invsum"].T.reshape(ROWS)

    # host finalize, mirroring the reference ops in f32 (touches 16K scalars)
    p_gt = numer / denom
    nll = -np.log(p_gt + np.float32(EPS))
    m = (numer > 0).astype(np.float32)
    nll3 = nll.reshape(B, S_TGT, L_TGT)
    m3 = m.reshape(B, S_TGT, L_TGT)
    nvalid = m3.sum(-1)
    seg_loss = np.where(
        nvalid > 0, (nll3 * m3).sum(-1) / np.maximum(nvalid, np.float32(1.0)), 0.0
    ).astype(np.float32)
    cnt = int((nvalid > 0).sum())
    total = np.float32(seg_loss.sum(dtype=np.float32) / np.float32(max(cnt, 1)))
    return np.asarray(total, np.float32), np.asarray(cnt, np.int32)



# revision 11
# speedup vs baseline: 1.9915x; 1.9915x over previous
"""Masked-softmax cross-entropy loss on 8 Trainium2 cores.

Math per target row t (16384 rows of length 4096):
  numer[t] = sum_j exp(x[t,j]/tau) over valid src cols j with color == tgt color t
  denom[t] = sum_j exp(x[t,j]/tau) over valid src cols j
  p_gt = numer/denom, nll = -log(p_gt + eps); rows with numer==0 are masked.
Segment aggregation (32 segments) happens on host - it touches 16K scalars.

Sharding: core c takes half a batch: batch c//2, row-half c%2 (2048 rows).

Device design (v2) - transposed layout + TensorE color buckets:
  The similarity map is shipped as f16 *transposed*: per core x^T is
  [4096 src j, 2048 tgt t], reshaped on host so chunk ci is a [128, 4096]
  DRAM slab-pair (partition = j within slab, free = slab-half * 2048 + t).
  Per chunk: SWDGE 1MB load -> ScalarE in-place exp(10*x) (f16) -> PE
  matmuls against a per-slab one-hot color matrix W [128 j, 99]:
     W[j, c] = 1 if src color id of j == c (c in 0..97), and
     W[j, 98] = 1 if j is a valid (non-pad) src column.
  PSUM accumulates bucket[c, t] over all 32 slabs; column 98 is the valid
  denominator. One DVE copy PSUM->SBUF and one output DMA per core.
  Host gathers numer[t] = bucket[tid[t], t], denom[t] = bucket[98, t].
  This keeps ScalarE (the exp engine, 1 elem/cycle/lane) as the only
  saturated engine: ~59us of exp vs ~142us of 1x-mode DVE STTs in v1.

Sync-wait budget: walrus allows very few sem waits per instruction
(1 for DMA/CTRL). Tiny pool-queue "interposer" copies absorb the
extra cross-engine waits ahead of each load; the kernel-tail drain is
split into one drain per proc.
"""

import os
import numpy as np

B = 4
S_TGT = 8
L_TGT = 512
C = 4
N = 4096          # src columns (= 8*512), also total tgt rows per batch
P = 128
ROWS = 2048       # tgt rows per core (half a batch)
NSLAB = N // P    # 32 j-slabs of 128 src columns
NCHUNK = NSLAB // 2   # 16 chunks; chunk = 2 slabs = [128, 4096] f16 = 1MB
NBUF = 4          # chunk buffer depth (slot reuse distance)
NID = 98          # color ids occupy 0..97 (97 palette colors + pad color)
MCOL = NID + 1    # one-hot columns: 98 id buckets + 1 valid-mask denom col
QCHUNK = 4        # PSUM t-chunks of 512 (one bank each)
NCORES = 8
PAD = -1.0
EPS = 1e-15

_NC_CACHE = {}


def _patch_split_drain():
    """Split the kernel-tail drain's sem waits across several drain
    instructions (walrus rejects >1 sync wait on one CTRL instruction)."""
    import concourse.tile as tile
    from concourse.vector_clock import ScopedClock, VectorClock

    if getattr(tile.TileContext, "_split_drain_patched", False):
        return

    def _drain_and_barrier(self, tick_clock, wait_clock):
        g = tick_clock.global_clock
        n = len(g)
        for base in range(n):
            vec = [g[i] if i == base else 0 for i in range(n)]
            if not any(vec):
                continue
            d = self.nc.sync.drain()
            wait_clock.add_sem_waits(d.ins, ScopedClock({None: VectorClock(vec)}))
        self.nc.all_engine_barrier()
        popped = self.nc._tile_sem_poison_stack.pop()
        assert popped is self._sem_poison
        self.nc.clear_and_free_semaphores(list(self.sems.allocated().values()))
        self.nc.all_engine_barrier()

    tile.TileContext._drain_and_barrier = _drain_and_barrier
    tile.TileContext._split_drain_patched = True


def _build_nc():
    import concourse.bass as bass
    import concourse.mybir as mybir
    import concourse.tile as tile
    from concourse.tile_rust import add_dep_helper
    from contextlib import ExitStack

    _patch_split_drain()
    nc = bass.Bass()
    f32 = mybir.dt.float32
    f16 = mybir.dt.float16
    bf16 = mybir.dt.bfloat16
    NW = 2 * N  # unused width marker (chunk free size is 4096)

    # x chunk layout: row 128*ci + p, col 2048*hh + t  <=>  x^T[j, t] with
    # j = 128*(2*ci + hh) + p  (host packs it this way)
    x = nc.declare_dram_parameter("x", [NCHUNK * P, 2 * ROWS], f16, isOutput=False)
    # w layout: [p, 99*s + c] = one-hot for src col j = 128*s + p
    w = nc.declare_dram_parameter("w", [P, NSLAB * MCOL], bf16, isOutput=False)
    buckets = nc.declare_dram_parameter("buckets", [MCOL, ROWS], f32, isOutput=True)

    with tile.TileContext(nc) as tc:
        with ExitStack() as ctx:
            const_pool = ctx.enter_context(tc.tile_pool(name="const", bufs=1))
            # x input tiles (f16, recycled): the recycle waits are absorbed
            # ahead of each load. et output tiles (bf16 - exp(10x) reaches
            # 7e23, far over f16 max, and the numerator needs range down to
            # ~e^-55) are never recycled: 16 x 8KB/partition = 128KB, so
            # each exp writes fresh SBUF and carries only its load's wait.
            x_pool = ctx.enter_context(tc.tile_pool(name="x", bufs=NBUF))
            data_pool = ctx.enter_context(tc.tile_pool(name="data", bufs=1))
            res_pool = ctx.enter_context(tc.tile_pool(name="res", bufs=1))
            psum_pool = ctx.enter_context(
                tc.tile_pool(name="psum", bufs=1, space="PSUM")
            )

            wt = const_pool.tile([P, NSLAB * MCOL], bf16)
            nc.sync.dma_start(wt[:], w[:])

            # absorber scratch: one column per (chunk, kind)
            scr = const_pool.tile([P, 3 * NCHUNK + 4], f16)
            warm = const_pool.tile([P, 2], f16)

            res = res_pool.tile([P, ROWS], f32)
            pt = psum_pool.tile([P, ROWS], f32)

            # warm-ups: absorb the w const-DMA wait into each engine's
            # clock so later absorbers reading wt carry only their explicit
            # dep. The ACT one doubles as the exp table-load warm-up.
            nc.scalar.activation(
                warm[:, 0:1], wt[:, 0:1], mybir.ActivationFunctionType.Exp,
                scale=10.0,
            )
            nc.gpsimd.tensor_copy(warm[:, 1:2], wt[:, 0:1])
            # PE absorber for the w const-DMA wait
            nc.tensor.ldweights(wt[:, 0:MCOL])

            load_insts = []
            mm_last = []
            for ci in range(NCHUNK):
                # pool-queue interposers: absorb the x-slot WAW (old load,
                # DMA sem) and the DMA sem-lane WAW so the DMACopy carries
                # only the x-slot WAR (ACT clock of exp(ci-NBUF))
                pre = []
                if ci >= NBUF:
                    a1 = nc.gpsimd.tensor_copy(
                        scr[:, 3 * ci:3 * ci + 1], wt[:, 0:1]
                    )
                    add_dep_helper(
                        a1.ins, load_insts[ci - NBUF].ins, sync=True,
                        reason="absorb x-slot WAW vs old load",
                    )
                    pre.append(a1)
                if ci >= 8:
                    a3 = nc.gpsimd.tensor_copy(
                        scr[:, 3 * ci + 2:3 * ci + 3], wt[:, 0:1]
                    )
                    add_dep_helper(
                        a3.ins, load_insts[ci - 8].ins, sync=True,
                        reason="absorb DMA lane WAW",
                    )
                    pre.append(a3)

                xt = x_pool.tile([P, 2 * ROWS], f16)
                ld = nc.gpsimd.dma_start(
                    xt[:], x[ci * P:(ci + 1) * P, :]
                )
                for g in pre:
                    add_dep_helper(
                        ld.ins, g.ins, sync=False,
                        reason="load ordered after wait absorber",
                    )
                load_insts.append(ld)

                # in-place exp: et == xt (f16 -> f16), no extra slot, so
                # the only sem wait on the ACTIVATE is the load's DMA sem.
                et = data_pool.tile([P, 2 * ROWS], bf16, name=f"et{ci}",
                                    tag=f"et{ci}")
                nc.scalar.activation(
                    et[:], xt[:], mybir.ActivationFunctionType.Exp,
                    scale=10.0,
                )

                last = None
                for hh in range(2):
                    s = 2 * ci + hh
                    for q in range(QCHUNK):
                        last = nc.tensor.matmul(
                            pt[0:MCOL, q * 512:(q + 1) * 512],
                            wt[:, s * MCOL:(s + 1) * MCOL],
                            et[:, hh * ROWS + q * 512:hh * ROWS + (q + 1) * 512],
                            start=(s == 0),
                            stop=(s == NSLAB - 1),
                            skip_group_check=True,
                        )
                mm_last.append(last)

            nc.vector.tensor_copy(res[0:MCOL, :], pt[0:MCOL, :])
            nc.sync.dma_start(buckets[:], res[0:MCOL, :])
    return nc


def _get_nc():
    key = (NBUF, NCHUNK)
    if key not in _NC_CACHE:
        _NC_CACHE[key] = _build_nc()
    return _NC_CACHE[key]


def _color_ids(src, tgt):
    """Map each color row to a per-batch integer id via exact byte equality."""
    src_f = np.ascontiguousarray(src.reshape(B, -1, C))
    tgt_f = np.ascontiguousarray(tgt.reshape(B, -1, C))
    n_s = src_f.shape[1]
    src_ids = np.empty((B, n_s), np.int32)
    tgt_ids = np.empty((B, tgt_f.shape[1]), np.int32)
    for b in range(B):
        allc = np.ascontiguousarray(np.concatenate([src_f[b], tgt_f[b]], axis=0))
        view = allc.view([("", allc.dtype)] * C).reshape(-1)
        _, inv = np.unique(view, return_inverse=True)
        ids = inv.astype(np.int32)
        s_ids, t_ids = ids[:n_s].copy(), ids[n_s:].copy()
        s_ids[np.all(src_f[b] == PAD, axis=-1)] = -1
        t_ids[np.all(tgt_f[b] == PAD, axis=-1)] = -2
        src_ids[b], tgt_ids[b] = s_ids, t_ids
    return src_ids, tgt_ids


def kernel(seg_sim_map, seg_colors_src, seg_colors_tgt):
    import ml_dtypes
    from concourse.bass_utils import run_bass_kernel_spmd

    seg_sim_map = np.asarray(seg_sim_map, dtype=np.float32)
    src_ids, tgt_ids = _color_ids(
        np.asarray(seg_colors_src, np.float32), np.asarray(seg_colors_tgt, np.float32)
    )
    assert src_ids.max() < NID and tgt_ids.max() < NID

    # per-batch one-hot W in the device layout [P, 32*99]
    w_dev = {}
    for b in range(B):
        onehot = np.zeros((N, MCOL), ml_dtypes.bfloat16)
        valid = src_ids[b] >= 0
        onehot[np.arange(N)[valid], src_ids[b][valid]] = 1.0
        onehot[valid, NID] = 1.0  # denom column: any valid src col
        # [N, MCOL] -> [NSLAB, P, MCOL] -> [P, NSLAB, MCOL] -> [P, NSLAB*MCOL]
        w_dev[b] = np.ascontiguousarray(
            onehot.reshape(NSLAB, P, MCOL).transpose(1, 0, 2).reshape(P, -1)
        )

    in_maps = []
    for c in range(NCORES):
        b, h = c // 2, c % 2
        rows = slice(h * ROWS, (h + 1) * ROWS)
        # x^T: [4096 j, 2048 t] -> chunks [16, 2, 128, 2048] ->
        # [16, 128, 2, 2048] -> [2048, 4096]
        xT = np.ascontiguousarray(seg_sim_map[b, rows, :].T.astype(np.float16))
        x_dev = np.ascontiguousarray(
            xT.reshape(NCHUNK, 2, P, ROWS).transpose(0, 2, 1, 3).reshape(
                NCHUNK * P, 2 * ROWS
            )
        )
        in_maps.append({"x": x_dev, "w": w_dev[b]})

    trace = os.environ.get("KERNEL_PROFILE", "") == "1"
    nc = _get_nc()
    out = run_bass_kernel_spmd(nc, in_maps, list(range(NCORES)), trace=trace)
    if trace and out.exec_time_ns is not None:
        print(f"HW exec time: {out.exec_time_ns} ns")
        print(f"HW exec mean: {out.mean_exec_time_ns} ns")

    numer = np.empty((B, N), np.float32)
    denom = np.empty((B, N), np.float32)
    for c in range(NCORES):
        b, h = c // 2, c % 2
        rows = slice(h * ROWS, (h + 1) * ROWS)
        buckets = out.results[c]["buckets"]            # [99, 2048]
        tid = tgt_ids[b, rows]
        gather = buckets[np.where(tid >= 0, tid, 0), np.arange(ROWS)]
        numer[b, rows] = np.where(tid >= 0, gather, 0.0)
        denom[b, rows] = buckets[NID]

    # host finalize, mirroring the reference ops in f32 (touches 16K scalars)
    p_gt = numer / denom
    nll = -np.log(p_gt + np.float32(EPS))
    m = (numer > 0).astype(np.float32)
    nll3 = nll.reshape(B, S_TGT, L_TGT)
    m3 = m.reshape(B, S_TGT, L_TGT)
    nvalid = m3.sum(-1)
    seg_loss = np.where(
        nvalid > 0, (nll3 * m3).sum(-1) / np.maximum(nvalid, np.float32(1.0)), 0.0
    ).astype(np.float32)
    cnt = int((nvalid > 0).sum())
    total = np.float32(seg_loss.sum(dtype=np.float32) / np.float32(max(cnt, 1)))
    return np.asarray(total, np.float32), np.asarray(cnt, np.int32)
